# revision 7
# baseline (speedup 1.0000x reference)
"""Based (2nd-order Taylor linear attention) Trainium2 kernel.

Problem: nn_Based_56719338111472.
  hidden [1, 512, 768] -> q,k (12 heads, f=16), v (12 heads, d=64)
  phi = 2nd-order taylor feature map (D = 1 + 16 + 256 = 273)
  causal linear attention, output projection Wo.

Key identity: phi(q)·phi(k) = 1 + (q·k)/4 + (q·k)^2/32 = 0.5 + ((q·k)+4)^2/32
so the entire feature map collapses to a polynomial on the plain q·k score —
we run exact block-causal quadratic attention with K=16 score matmuls and a
Square activation, never materializing the 273-dim features.

Sharding: head-parallel, 2 heads per core (heads 2c, 2c+1; heads >= 12 are
zero-padded virtual heads). Each core computes outT_partial = Wo_blk.T @ y_blk
(row-parallel proj_o); the host sums the per-core partials (the unshard step
for a row-parallel sharding) and transposes back.

All device matmuls run in bf16 with fp32 PSUM accumulation; the score
polynomial and reciprocal run in fp32.
"""

import math

import ml_dtypes
import numpy as np

import concourse.bass as bass
import concourse.tile as tile
from concourse import bacc, mybir
from concourse.bass import ts
from concourse.bass_utils import run_bass_kernel_spmd

# ---- problem constants (hardcoded; kernel.py must be self-contained) ----
L = 512          # sequence length
E = 768          # d_model
F = 16           # feature dim per head
HD = 64          # head dim (v)
NH = 12          # real heads
C = 128          # chunk size
NCH = L // C     # 4 chunks
ECH = E // 128   # 6 e-chunks
NCORES = 8
HPC = 2          # heads per core (16 virtual heads)

_SQ_SCALE = 1.0 / math.sqrt(32.0)
_SQ_BIAS = 4.0 / math.sqrt(32.0)

BF16 = mybir.dt.bfloat16
F32 = mybir.dt.float32


def build_kernel():
    """Build and compile the per-core Bass program (identical on all cores)."""
    nc = bacc.Bacc("TRN2", debug=False, enable_asserts=False)

    ht_d = nc.dram_tensor("ht", (E, L), BF16, kind="ExternalInput").ap()
    wq_d = nc.dram_tensor("wq", (E, 80), BF16, kind="ExternalInput").ap()
    wk_d = nc.dram_tensor("wk", (E, 80), BF16, kind="ExternalInput").ap()
    wv_d = nc.dram_tensor("wv", (E, 128), BF16, kind="ExternalInput").ap()
    wo_d = nc.dram_tensor("wo", (128, E), BF16, kind="ExternalInput").ap()
    mask_d = nc.dram_tensor("maskT", (C, C), BF16, kind="ExternalInput").ap()
    outp_d = nc.dram_tensor("outp", (E, L), BF16, kind="ExternalOutput").ap()

    ht_r = ht_d.rearrange("(e p) m -> p e m", p=128)

    with tile.TileContext(nc) as tc:
        with (
            tc.tile_pool(name="const", bufs=1) as const_pool,
            tc.tile_pool(name="work", bufs=1) as work,
            tc.tile_pool(name="sq_p", bufs=3) as sq_pool,
            tc.tile_pool(name="sc_p", bufs=4) as sc_pool,
            tc.tile_pool(name="o_p", bufs=3) as o_pool,
            tc.tile_pool(name="ps1", bufs=1, space="PSUM") as ps1,
            tc.tile_pool(name="ps_blk", bufs=3, space="PSUM") as ps_blk,
        ):
            # ---- input loads, split across the two HWDGE rings ----
            wq_sb = const_pool.tile([128, ECH, 80], BF16, name="wq_sb")
            nc.sync.dma_start(wq_sb, wq_d.rearrange("(e p) c -> p e c", p=128))
            wk_sb = const_pool.tile([128, ECH, 80], BF16, name="wk_sb")
            nc.sync.dma_start(wk_sb, wk_d.rearrange("(e p) c -> p e c", p=128))
            ht_sb = const_pool.tile([128, ECH, L], BF16, name="ht_sb")
            for e in range(3):
                nc.sync.dma_start(ht_sb[:, e, :], ht_r[:, e, :])
            wv_sb = const_pool.tile([128, ECH, 128], BF16, name="wv_sb")
            nc.scalar.dma_start(wv_sb, wv_d.rearrange("(e p) c -> p e c", p=128))
            for e in range(3, ECH):
                nc.scalar.dma_start(ht_sb[:, e, :], ht_r[:, e, :])
            wo_sb = const_pool.tile([128, E], BF16, name="wo_sb")
            nc.scalar.dma_start(wo_sb, wo_d)
            mask_sb = const_pool.tile([C, C], BF16, name="mask_sb")
            nc.scalar.dma_start(mask_sb, mask_d)

            # ---- constants ----
            ones_sb = const_pool.tile([128, 128], BF16, name="ones_sb")
            nc.vector.memset(ones_sb, 1.0)
            sqbias_sb = const_pool.tile([128, 1], F32, name="sqbias_sb")
            nc.vector.memset(sqbias_sb, _SQ_BIAS)
            y_sb = work.tile([128, L], BF16, name="y_sb")
            nc.vector.memset(y_sb, 0.0)
            # tiny dummy activation: forces the ACT LUT table load to overlap
            # the input DMA phase instead of the first real Square
            dummy_sb = const_pool.tile([1, 1], F32, name="dummy_sb")
            nc.scalar.activation(
                dummy_sb,
                sqbias_sb[0:1, :],
                mybir.ActivationFunctionType.Square,
                bias=sqbias_sb[0:1, :],
                scale=1.0,
            )

            ps_num = ps1.tile([128, L], F32, name="ps_num")
            ps_den = ps1.tile([128, L], F32, name="ps_den")
            ps_v = ps1.tile([128, NCH, 128], F32, name="ps_v")
            ps_q = ps1.tile([128, L], F32, name="ps_q")
            ps_k = ps1.tile([128, L], F32, name="ps_k")

            # ---- PE warm-up during the DMA wait: ones @ zeros matmuls ----
            # These zero-fill ps_num/ps_den/ps_v (replacing memsets — the
            # later accumulation runs with start=False) and keep the PE busy
            # so the HAM clock-gate opens before the real projections start.
            nc.tensor.matmul(
                ps_num, ones_sb, y_sb, start=True, stop=True, skip_group_check=True
            )
            nc.tensor.matmul(
                ps_den, ones_sb, y_sb, start=True, stop=True, skip_group_check=True
            )
            nc.tensor.matmul(
                ps_v[:, :, :], ones_sb, y_sb, start=True, stop=True,
                skip_group_check=True,
            )
            for _ in range(5):
                nc.tensor.matmul(
                    ps_q, ones_sb, y_sb, start=True, stop=True, skip_group_check=True
                )

            # ---- q/k projections ----
            # lhsT [e-chunk, 80] -> psum rows 0:16 = head0 qT/kT, rows
            # 64:80 = head1 (32-aligned bases for the K=16 score matmuls).
            for e in range(ECH):
                st, sp = (e == 0), (e == ECH - 1)
                nc.tensor.matmul(
                    ps_q[0:80, :], wq_sb[:, e, :], ht_sb[:, e, :], start=st, stop=sp
                )
                nc.tensor.matmul(
                    ps_k[0:80, :], wk_sb[:, e, :], ht_sb[:, e, :], start=st, stop=sp
                )
            q_sb = work.tile([80, L], BF16, name="q_sb")
            nc.vector.tensor_copy(q_sb, ps_q[0:80, :])
            k_sb = work.tile([80, L], BF16, name="k_sb")
            nc.scalar.copy(k_sb, ps_k[0:80, :])

            # ---- attention score blocks (interleave heads for overlap) ----
            # scT[n, m] = 0.5 + ((S0 + 4)^2)/32, causal-masked on the diagonal
            sc_tiles = {}
            for j in range(NCH):
                for h in range(HPC):
                    b = 64 * h
                    nj = L - C * j
                    ps_s = ps_blk.tile([128, L], F32, name="ps_s", tag="blk")
                    nc.tensor.matmul(
                        ps_s[:, 0:nj],
                        k_sb[b : b + F, ts(j, C)],
                        q_sb[b : b + F, C * j : L],
                        start=True,
                        stop=True,
                    )
                    sq = sq_pool.tile([128, L], BF16, name="sq")
                    nc.scalar.activation(
                        sq[:, 0:nj],
                        ps_s[:, 0:nj],
                        mybir.ActivationFunctionType.Square,
                        bias=sqbias_sb[:, :],
                        scale=_SQ_SCALE,
                    )
                    scT = sc_pool.tile([128, L], BF16, name="scT")
                    nc.vector.scalar_tensor_tensor(
                        scT[:, 0:C],
                        sq[:, 0:C],
                        0.5,
                        mask_sb,
                        op0=mybir.AluOpType.add,
                        op1=mybir.AluOpType.mult,
                    )
                    if nj > C:
                        nc.vector.tensor_scalar_add(scT[:, C:nj], sq[:, C:nj], 0.5)
                    sc_tiles[(h, j)] = scT

            # ---- v projection, directly in [n, d] layout ----
            # v[n, d] = sum_e ht[e, n] * WvT[e, d]: lhsT = ht chunk (reused
            # weights), rhs = WvT. No DMA transposes needed.
            v_sb = work.tile([128, NCH, 128], BF16, name="v_sb")
            for i in range(NCH):
                for e in range(ECH):
                    nc.tensor.matmul(
                        ps_v[:, i, :],
                        ht_sb[:, e, ts(i, C)],
                        wv_sb[:, e, :],
                        start=False,
                        stop=(e == ECH - 1),
                        skip_group_check=True,
                    )
                nc.vector.tensor_copy(v_sb[:, i, :], ps_v[:, i, :])

            # ---- num/den accumulation over blocks ----
            for j in range(NCH):
                for h in range(HPC):
                    b = 64 * h
                    nj = L - C * j
                    scT = sc_tiles[(h, j)]
                    nc.tensor.matmul(
                        ps_num[b : b + HD, C * j : L],
                        v_sb[:, j, b : b + HD],
                        scT[:, 0:nj],
                        start=False,
                        stop=(j == NCH - 1),
                        skip_group_check=True,
                    )
                    nc.tensor.matmul(
                        ps_den[b : b + HD, C * j : L],
                        ones_sb[:, 0:HD],
                        scT[:, 0:nj],
                        start=False,
                        stop=(j == NCH - 1),
                        skip_group_check=True,
                    )

            # ---- divide: y = num / den (den rows replicated per head) ----
            rden_sb = work.tile([128, L], F32, name="rden_sb")
            nc.vector.reciprocal_approx_fast(rden_sb, ps_den)
            nc.vector.tensor_mul(y_sb, ps_num, rden_sb)

            # ---- output projection (row-parallel partial, bf16 out) ----
            H = L // 2
            for cc in range(ECH):
                ps_o = ps_blk.tile([128, L], F32, name="ps_o", tag="blk")
                nc.tensor.matmul(
                    ps_o, wo_sb[:, ts(cc, C)], y_sb, start=True, stop=True
                )
                o_sb = o_pool.tile([128, L], BF16, name="o_sb")
                # evacuate + store in halves to shorten the tail
                for half in range(2):
                    sl = slice(half * H, (half + 1) * H)
                    if (cc + half) % 2 == 0:
                        nc.vector.tensor_copy(o_sb[:, sl], ps_o[:, sl])
                    else:
                        nc.scalar.copy(o_sb[:, sl], ps_o[:, sl])
                    eng = nc.sync if (cc + half) % 2 == 0 else nc.scalar
                    eng.dma_start(outp_d[ts(cc, C), sl], o_sb[:, sl])

    nc.compile()
    return nc


def make_core_inputs(hidden_states, Wq, Wk, Wv, Wo):
    """Host-side marshalling: transpose/cast/shard the full inputs."""
    bf16 = ml_dtypes.bfloat16
    h = np.ascontiguousarray(hidden_states[0].T).astype(bf16)  # [768, 512]
    maskT = np.triu(np.ones((C, C), np.float32)).astype(bf16)  # keep n <= m

    WqT = Wq.astype(np.float32).T  # [768, 192]
    WkT = Wk.astype(np.float32).T
    WvT = Wv.astype(np.float32).T  # [768, 768]
    # Wo: out = y_flat @ Wo.T ; per-core rows block = Wo[:, hd_cols].T
    in_maps = []
    for c in range(NCORES):
        wq = np.zeros((E, 80), np.float32)
        wk = np.zeros((E, 80), np.float32)
        wv = np.zeros((E, 128), np.float32)
        wo = np.zeros((128, E), np.float32)
        for hh in range(HPC):
            head = HPC * c + hh
            if head >= NH:
                continue
            b = 64 * hh
            wq[:, b : b + F] = WqT[:, F * head : F * (head + 1)]
            wk[:, b : b + F] = WkT[:, F * head : F * (head + 1)]
            wv[:, 64 * hh : 64 * hh + HD] = WvT[:, HD * head : HD * (head + 1)]
            wo[64 * hh : 64 * hh + HD, :] = Wo[:, HD * head : HD * (head + 1)].T
        in_maps.append(
            {
                "ht": h,
                "wq": wq.astype(bf16),
                "wk": wk.astype(bf16),
                "wv": wv.astype(bf16),
                "wo": wo.astype(bf16),
                "maskT": maskT,
            }
        )
    return in_maps


_NC_CACHE = {}


def kernel(hidden_states, Wq, Wk, Wv, Wo):
    if "nc" not in _NC_CACHE:
        _NC_CACHE["nc"] = build_kernel()
    nc = _NC_CACHE["nc"]
    in_maps = make_core_inputs(hidden_states, Wq, Wk, Wv, Wo)
    res = run_bass_kernel_spmd(nc, in_maps, core_ids=list(range(NCORES)))
    ncores_real = (NH + HPC - 1) // HPC  # cores that hold real heads
    acc = np.zeros((E, L), np.float64)
    for c in range(ncores_real):
        acc += res.results[c]["outp"].astype(np.float64)
    out = acc.T.astype(np.float32).reshape(1, L, E)
    return out
